# revision 1
# baseline (speedup 1.0000x reference)
"""2-layer GAT (GATConv x2, PyG-style) on 8 Trainium2 NeuronCores.

Strategy (dst-node sharding, edge/graph parallelism):
  - Self-loops appended; edges sorted by dst. Core c owns dst nodes
    [c*NS, (c+1)*NS) and every edge pointing into that range, so the
    segment softmax / scatter-reduce needs no cross-core reduction.
  - Per layer, a node phase computes h = x @ W plus per-node attention
    logits (fused via a block-diagonal attention matrix) and writes a
    bf16 gather table row per node; tables are AllGathered so any core
    can fetch rows for arbitrary src ids.
  - Edge phase: edges are packed into tiles (<=128 dst nodes, 16
    128-edge blocks). Rows are fetched with the hardware bulk gather
    (dma_gather, int16 indices) -- the 100k-row table is split into 4
    chunks of 25k rows and each tile reserves a fixed 4-block quota
    per chunk. Per-edge weights w = exp(leaky_relu(s_src+s_dst)); a
    0/1 one-hot [edge, local_dst] built on the vector engine routes
    weighted messages into PSUM via TensorE matmuls (segment-sum as
    matmul). Denominators ride along as a per-head "ones" column, so
    softmax normalization is one reciprocal+scale per node.
  - All per-core variation (tile node ranges, edge indices, padding)
    lives in data/index arrays so one SPMD program serves all 8 cores.
"""

import math
import numpy as np

import concourse.bass as bass
import concourse.bacc as bacc
import concourse.tile as tile
from concourse import mybir
from concourse.bass import IndirectOffsetOnAxis, AP
from concourse.masks import make_identity

F32 = mybir.dt.float32
BF16 = mybir.dt.bfloat16
I32 = mybir.dt.int32
I16 = mybir.dt.int16

# Full problem constants
N_NODES = 100000
N_EDGES = 1600000
IN_CH = 128
HID = 32
HEADS = 4
NEG_SLOPE = 0.2
N_CORES = 8

# padded rows store into a dummy row (index NS) instead of using the
# hardware bounds check, which proved unsafe at scale
NQ = 4             # src chunks (table rows per chunk must fit int16)


class Cfg:
    def __init__(self, n_nodes=N_NODES, n_edges=N_EDGES, n_cores=N_CORES, nbq=4):
        assert n_nodes % n_cores == 0 and n_nodes % NQ == 0
        self.N = n_nodes
        self.E = n_edges
        self.C = n_cores
        self.NS = n_nodes // n_cores   # nodes per core (dst shard)
        self.CH = n_nodes // NQ        # table chunk rows
        assert self.CH < 32768
        self.NBQ = nbq                 # 128-edge blocks reserved per src chunk
        self.G = NQ * nbq              # blocks per tile
        self.SLOTS = self.G * 128
        self.QS = nbq * 128            # slots per quarter
        # bf16 table rows (256B gather granularity)
        self.R1 = 256   # [h0,1,h1,1,h2,1,h3,1, s_src(4), pad] bf16
        self.R2 = 128   # [h2(32), 1, s2_src, pad] bf16
        self.RS = 128   # s_dst table row (bf16; 4 / 1 cols used)


# ---------------------------------------------------------------------------
# Host-side preprocessing
# ---------------------------------------------------------------------------

def preprocess(edge_index, cfg: Cfg):
    src = np.ascontiguousarray(edge_index[0]).astype(np.int64)
    dst = np.ascontiguousarray(edge_index[1]).astype(np.int64)
    loops = np.arange(cfg.N, dtype=np.int64)
    src = np.concatenate([src, loops]).astype(np.int32)
    dst = np.concatenate([dst, loops]).astype(np.int32)
    chunk = src // cfg.CH

    # sort by (chunk, dst): per chunk a dst-sorted stream
    order = np.lexsort((dst, chunk))
    src, dst, chunk = src[order], dst[order], chunk[order]
    q_starts = np.searchsorted(chunk, np.arange(NQ + 1))
    # per-chunk dst arrays for range queries
    dst_q = [dst[q_starts[q]:q_starts[q + 1]] for q in range(NQ)]
    src_q = [src[q_starts[q]:q_starts[q + 1]] for q in range(NQ)]

    # per-node per-chunk degree
    cnt_nq = np.zeros((cfg.N, NQ), dtype=np.int64)
    np.add.at(cnt_nq, (dst, chunk), 1)

    per_core_tiles = []
    for c in range(cfg.C):
        n_lo, n_hi = c * cfg.NS, (c + 1) * cfg.NS
        tiles = []
        n = n_lo
        while n < n_hi:
            t_lo = n
            qfill = np.zeros(NQ, dtype=np.int64)
            while n < n_hi and n - t_lo < 128:
                nxt = qfill + cnt_nq[n]
                if (nxt > cfg.QS).any():
                    break
                qfill = nxt
                n += 1
            if n == t_lo:
                raise ValueError(f"node {t_lo} too high degree for quota")
            tiles.append((t_lo, n))
        per_core_tiles.append(tiles)

    T = max(len(t) for t in per_core_tiles)

    C, G, S = cfg.C, cfg.G, cfg.SLOTS
    # dma_gather index arrays, pre-replicated to 128 partitions (the Q7
    # cores each read their own 16-partition group)
    gidx = np.zeros((C, T, NQ, 128, cfg.QS // 16), dtype=np.int16)
    sidx = np.zeros((C, T, 128, S // 16), dtype=np.int16)
    dst_local = np.full((C, T, 128, G), -1, dtype=np.int32)
    out_idx = np.full((C, T, 128), cfg.NS, dtype=np.int32)

    def wrap16(vals, n):
        # element i at [i%16, i//16], replicated to 128 partitions
        grid = vals.reshape(n // 16, 16).T
        return np.tile(grid, (8, 1))

    for c in range(cfg.C):
        base = c * cfg.NS
        for t, (nlo, nhi) in enumerate(per_core_tiles[c]):
            nn = nhi - nlo
            out_idx[c, t, :nn] = np.arange(nlo - base, nhi - base, dtype=np.int32)
            stmp = np.zeros(S, dtype=np.int16)
            for q in range(NQ):
                lo = np.searchsorted(dst_q[q], nlo)
                hi = np.searchsorted(dst_q[q], nhi)
                cnt = hi - lo
                assert cnt <= cfg.QS
                e_src = src_q[q][lo:hi]
                e_dst = dst_q[q][lo:hi]
                i = np.arange(cnt)
                # slot i of quarter q -> partition i%128, block q*NBQ + i//128
                gi = np.zeros(cfg.QS, dtype=np.int16)
                gi[:cnt] = (e_src - q * cfg.CH).astype(np.int16)
                gidx[c, t, q] = wrap16(gi, cfg.QS)
                blk = q * cfg.NBQ + i // 128
                par = i % 128
                dst_local[c, t, par, blk] = e_dst - nlo
                stmp[blk * 128 + par] = (e_dst - base).astype(np.int16)
            sidx[c, t] = wrap16(stmp, S)
    return dict(gidx=gidx, sidx=sidx, dst_local=dst_local,
                out_idx=out_idx, n_tiles=T)


def make_blockdiag(att_src, att_dst):
    heads, hid = att_src.shape
    A = np.zeros((heads * hid, 2 * heads), dtype=np.float32)
    for h in range(heads):
        A[h * hid:(h + 1) * hid, h] = att_src[h]
        A[h * hid:(h + 1) * hid, heads + h] = att_dst[h]
    return A


# ---------------------------------------------------------------------------
# Device program
# ---------------------------------------------------------------------------

def bcast_mid(ap: AP, reps: int) -> AP:
    (p_step, p_num), rest = ap.ap[0], list(ap.ap[1:])
    return AP(tensor=ap.tensor, offset=ap.offset,
              ap=[[p_step, p_num], [0, reps]] + rest)


def build_program(cfg: Cfg, n_tiles: int, phases=(1, 2, 3), cap2=None, cap3=None):
    from contextlib import ExitStack
    nc = bacc.Bacc(None, target_bir_lowering=False)
    C, G, NS, R1, R2, RS = cfg.C, cfg.G, cfg.NS, cfg.R1, cfg.R2, cfg.RS
    AW = 2 * HEADS
    HB = HEADS * (HID + 1)  # 132
    NT1 = math.ceil(NS / 128)

    # ---- IO ----
    x_in = nc.dram_tensor("x_shard", [NS, IN_CH], F32, kind="ExternalInput")
    W1_in = nc.dram_tensor("W1", [IN_CH, HEADS * HID], F32, kind="ExternalInput")
    A1_in = nc.dram_tensor("A1", [HEADS * HID, AW], F32, kind="ExternalInput")
    b1_in = nc.dram_tensor("bias1", [1, HEADS * HID], F32, kind="ExternalInput")
    W2_in = nc.dram_tensor("W2", [HEADS * HID, HID], F32, kind="ExternalInput")
    A2_in = nc.dram_tensor("A2", [HID, 2], F32, kind="ExternalInput")
    b2_in = nc.dram_tensor("bias2", [1, HID], F32, kind="ExternalInput")
    gidx_in = nc.dram_tensor("gidx", [n_tiles, NQ, 128, cfg.QS // 16], I16,
                             kind="ExternalInput")
    sidx_in = nc.dram_tensor("sidx", [n_tiles, 128, cfg.SLOTS // 16], I16,
                             kind="ExternalInput")
    dloc_in = nc.dram_tensor("dst_local", [n_tiles, 128, G], I32,
                             kind="ExternalInput")
    oidx_in = nc.dram_tensor("out_idx", [n_tiles, 128, 1], I32,
                             kind="ExternalInput")
    z_out = nc.dram_tensor("z", [NS + 128, HID], F32, kind="ExternalOutput")

    # ---- internal DRAM ----
    tab1_loc = nc.dram_tensor("tab1_loc", [NS, R1], BF16)
    tab1 = nc.dram_tensor("tab1", [cfg.N, R1], BF16, addr_space="Shared")
    sdst1 = nc.dram_tensor("sdst1", [NS, RS], BF16)
    tab2_loc = nc.dram_tensor("tab2_loc", [NS + 128, R2], BF16)
    tab2 = nc.dram_tensor("tab2", [cfg.N, R2], BF16, addr_space="Shared")
    sdst2 = nc.dram_tensor("sdst2", [NS + 128, RS], BF16)
    scratch_c2 = nc.dram_tensor("scratch_c2", [1, HID + 2], F32)

    replica_groups = [list(range(C))]

    with tile.TileContext(nc) as tc, ExitStack() as stack:
        consts = stack.enter_context(tc.tile_pool(name="consts", bufs=1))
        ppre_cm = tc.tile_pool(name="ppre", bufs=1, space="PSUM")
        ppre = ppre_cm.__enter__()

        identity = consts.tile([128, 128], F32)
        make_identity(nc, identity[:])
        iota_t = consts.tile([128, 128], I32)
        nc.gpsimd.iota(iota_t[:], pattern=[[1, 128]], base=0, channel_multiplier=0)

        # rhsW1 = [W1 | W1 @ A1]  [128, 136]
        rhsW1 = consts.tile([128, IN_CH + AW], F32)
        nc.sync.dma_start(out=rhsW1[:, :HEADS * HID], in_=W1_in[:])
        W1s = consts.tile([128, HEADS * HID], F32)
        nc.sync.dma_start(out=W1s[:], in_=W1_in[:])
        A1s = consts.tile([HEADS * HID, AW], F32)
        nc.sync.dma_start(out=A1s[:], in_=A1_in[:])
        w1t_ps = ppre.tile([128, 128], F32)
        nc.tensor.transpose(out=w1t_ps[:], in_=W1s[:], identity=identity[:])
        W1T = consts.tile([128, 128], F32)
        nc.scalar.copy(W1T[:], w1t_ps[:])
        w1a_ps = ppre.tile([128, AW], F32)
        nc.tensor.matmul(w1a_ps[:], lhsT=W1T[:], rhs=A1s[:], start=True, stop=True)
        nc.scalar.copy(rhsW1[:, IN_CH:], w1a_ps[:])

        # rhsW2 = [W2 | W2 @ A2]  [128, 34]
        rhsW2 = consts.tile([128, HID + 2], F32)
        W2s = consts.tile([128, HID], F32)
        nc.sync.dma_start(out=W2s[:], in_=W2_in[:])
        nc.sync.dma_start(out=rhsW2[:, :HID], in_=W2_in[:])
        A2s = consts.tile([HID, 2], F32)
        nc.sync.dma_start(out=A2s[:], in_=A2_in[:])
        w2t_ps = ppre.tile([HID, 128], F32)
        nc.tensor.transpose(out=w2t_ps[:], in_=W2s[:], identity=identity[:])
        W2T = consts.tile([HID, 128], F32)
        nc.scalar.copy(W2T[:], w2t_ps[:])
        w2a_ps = ppre.tile([128, 2], F32)
        nc.tensor.matmul(w2a_ps[:], lhsT=W2T[:], rhs=A2s[:], start=True, stop=True)
        nc.scalar.copy(rhsW2[:, HID:], w2a_ps[:])

        # c2 = column sums of rhsW2 (for the elu "-1" correction)
        ones_col = consts.tile([128, 1], F32)
        nc.vector.memset(ones_col[:], 1.0)
        c2_ps = ppre.tile([1, HID + 2], F32)
        nc.tensor.matmul(c2_ps[:], lhsT=ones_col[:], rhs=rhsW2[:], start=True,
                         stop=True)
        c2_row = consts.tile([1, HID + 2], F32)
        nc.vector.tensor_copy(c2_row[:], c2_ps[:])
        nc.sync.dma_start(out=scratch_c2[:], in_=c2_row[:])
        c2_b = consts.tile([128, HID + 2], F32)
        nc.sync.dma_start(out=c2_b[:],
                          in_=scratch_c2.ap().to_broadcast([128, HID + 2]))

        b1_b = consts.tile([128, HEADS * HID], F32)
        nc.sync.dma_start(out=b1_b[:], in_=b1_in.ap().to_broadcast([128, HEADS * HID]))
        b2_b = consts.tile([128, HID], F32)
        nc.sync.dma_start(out=b2_b[:], in_=b2_in.ap().to_broadcast([128, HID]))

        ppre_cm.__exit__(None, None, None)

        # ------------------------------------------------------------------
        # Phase A: node phase layer 1 -> tab1_loc, sdst1
        # ------------------------------------------------------------------
        with tc.tile_pool(name="pa", bufs=3) as pa, \
             tc.tile_pool(name="pa_ps", bufs=2, space="PSUM") as pa_ps:
            for it in range(NT1):
                lo = it * 128
                rows = min(128, NS - lo)
                x_t = pa.tile([128, IN_CH], F32, tag="x")
                nc.sync.dma_start(out=x_t[:rows], in_=x_in[lo:lo + rows, :])
                xt_ps = pa_ps.tile([128, 128], F32, tag="xt")
                nc.tensor.transpose(out=xt_ps[:], in_=x_t[:], identity=identity[:])
                xT = pa.tile([128, 128], F32, tag="xT")
                nc.scalar.copy(xT[:], xt_ps[:])
                hs_ps = pa_ps.tile([128, IN_CH + AW], F32, tag="hs")
                nc.tensor.matmul(hs_ps[:], lhsT=xT[:], rhs=rhsW1[:],
                                 start=True, stop=True)
                aug = pa.tile([128, R1], BF16, tag="aug")
                nc.vector.memset(aug[:, HB + HEADS:], 0.0)
                aug_v = aug[:, :HB].rearrange("p (h c) -> p h c", h=HEADS, c=HID + 1)
                hs_v = hs_ps[:, :HEADS * HID].rearrange(
                    "p (h c) -> p h c", h=HEADS, c=HID)
                nc.vector.tensor_copy(aug_v[:, :, :HID], hs_v)
                nc.vector.memset(aug_v[:, :, HID], 1.0)
                nc.scalar.copy(aug[:, HB:HB + HEADS],
                               hs_ps[:, HEADS * HID:HEADS * HID + HEADS])
                nc.sync.dma_start(out=tab1_loc[lo:lo + rows, :], in_=aug[:rows])
                sd = pa.tile([128, RS], BF16, tag="sd")
                nc.vector.memset(sd[:, HEADS:], 0.0)
                nc.scalar.copy(sd[:, :HEADS], hs_ps[:, HEADS * HID + HEADS:])
                nc.sync.dma_start(out=sdst1[lo:lo + rows, :], in_=sd[:rows])

        nc.gpsimd.collective_compute(
            "AllGather", mybir.AluOpType.bypass, replica_groups=replica_groups,
            ins=[tab1_loc.ap()], outs=[tab1.ap()])

        # ------------------------------------------------------------------
        # Phase C: edge phase layer 1 (+ fused layer-2 node phase)
        # ------------------------------------------------------------------
        if 2 not in phases:
            n_tiles_c = 0
        else:
            n_tiles_c = min(n_tiles, cap2) if cap2 else n_tiles
        tab1_q = [tab1[q * cfg.CH:(q + 1) * cfg.CH, :] for q in range(NQ)]
        with tc.tile_pool(name="pi", bufs=4) as pi, \
             tc.tile_pool(name="pg", bufs=3) as pg, \
             tc.tile_pool(name="po", bufs=3) as po, \
             tc.tile_pool(name="ps", bufs=3) as psm, \
             tc.tile_pool(name="pe_ps", bufs=3, space="PSUM") as pe_ps, \
             tc.tile_pool(name="pe_ps2", bufs=2, space="PSUM") as pe_ps2:
            for t in range(n_tiles_c):
                gi = pi.tile([128, NQ, cfg.QS // 16], I16, tag="gi")
                nc.sync.dma_start(
                    out=gi[:], in_=gidx_in[t].rearrange("q p s -> p q s"))
                si = pi.tile([128, cfg.SLOTS // 16], I16, tag="si")
                nc.sync.dma_start(out=si[:], in_=sidx_in[t])
                dloc = pi.tile([128, G], I32, tag="dloc")
                nc.sync.dma_start(out=dloc[:], in_=dloc_in[t])
                oidx = pi.tile([128, 1], I32, tag="oidx")
                nc.sync.dma_start(out=oidx[:], in_=oidx_in[t])

                hg = pg.tile([128, G, R1], BF16, tag="hg")
                for q in range(NQ):
                    nc.gpsimd.dma_gather(
                        out_ap=hg[:, q * cfg.NBQ:(q + 1) * cfg.NBQ, :],
                        in_ap=tab1_q[q],
                        idxs_ap=gi[:, q, :],
                        num_idxs=cfg.QS, num_idxs_reg=cfg.QS,
                        elem_size=R1)
                sde = pg.tile([128, G, RS], BF16, tag="sde")
                # <=1024 indices per call (SWDGE descriptor ring capacity;
                # exceeding it wedges the device)
                nsp = cfg.SLOTS // 1024 if cfg.SLOTS > 1024 else 1
                bsp = G // nsp
                assert bsp * 128 <= 1024 and bsp * nsp == G, (cfg.SLOTS, G)
                for hsp in range(nsp):
                    nc.gpsimd.dma_gather(
                        out_ap=sde[:, hsp * bsp:(hsp + 1) * bsp, :],
                        in_ap=sdst1.ap(),
                        idxs_ap=si[:, hsp * (bsp * 8):(hsp + 1) * (bsp * 8)],
                        num_idxs=bsp * 128, num_idxs_reg=bsp * 128, elem_size=RS)

                # w = exp(leaky_relu(s_src + s_dst))
                lg = psm.tile([128, G, HEADS], BF16, tag="lg")
                nc.vector.tensor_add(lg[:], hg[:, :, HB:HB + HEADS],
                                     sde[:, :, :HEADS])
                lr = psm.tile([128, G, HEADS], BF16, tag="lr")
                nc.vector.scalar_tensor_tensor(
                    out=lr[:], in0=lg[:], scalar=float(NEG_SLOPE), in1=lg[:],
                    op0=mybir.AluOpType.mult, op1=mybir.AluOpType.max)
                w_t = psm.tile([128, G, HEADS], BF16, tag="w")
                nc.scalar.activation(w_t[:], lr[:], mybir.ActivationFunctionType.Exp)

                # one-hot [edge, local dst]
                oh = po.tile([128, G, 128], BF16, tag="oh")
                nc.vector.tensor_tensor(
                    out=oh[:], in0=dloc[:].to_broadcast([128, G, 128]),
                    in1=bcast_mid(iota_t[:], G), op=mybir.AluOpType.is_equal)

                # weighted messages [h*w | w] per head
                rhs_b = pg.tile([128, G, HB], BF16, tag="rhsb")
                rhs_v = rhs_b[:].rearrange("p g (h c) -> p g h c", h=HEADS,
                                           c=HID + 1)
                hg_v = hg[:, :, :HB].rearrange("p g (h c) -> p g h c", h=HEADS,
                                               c=HID + 1)
                nc.vector.tensor_mul(rhs_v, hg_v,
                                     w_t[:].to_broadcast([128, G, HEADS, HID + 1]))

                acc_ps = pe_ps.tile([128, HB], F32, tag="acc")
                for g in range(G):
                    nc.tensor.matmul(acc_ps[:], lhsT=oh[:, g, :], rhs=rhs_b[:, g, :],
                                     start=(g == 0), stop=(g == G - 1))

                # normalize + bias + elu -> h1raw (true h1 = h1raw - 1)
                acc_v = acc_ps[:].rearrange("p (h c) -> p h c", h=HEADS, c=HID + 1)
                den = psm.tile([128, HEADS], F32, tag="den")
                nc.vector.tensor_scalar(out=den[:], in0=acc_v[:, :, HID],
                                        scalar1=1e-20, scalar2=None,
                                        op0=mybir.AluOpType.add)
                rec = psm.tile([128, HEADS], F32, tag="rec")
                nc.vector.reciprocal(rec[:], den[:])
                x1 = psm.tile([128, HEADS * HID], F32, tag="x1")
                x1_v = x1[:].rearrange("p (h c) -> p h c", h=HEADS, c=HID)
                for h in range(HEADS):
                    nc.vector.tensor_scalar(
                        out=x1_v[:, h, :], in0=acc_v[:, h, :HID],
                        scalar1=rec[:, h:h + 1], scalar2=None,
                        op0=mybir.AluOpType.mult)
                nc.vector.tensor_add(x1[:], x1[:], b1_b[:])
                mn = psm.tile([128, HEADS * HID], F32, tag="mn")
                nc.vector.tensor_scalar(out=mn[:], in0=x1[:], scalar1=0.0,
                                        scalar2=None, op0=mybir.AluOpType.min)
                ex = psm.tile([128, HEADS * HID], F32, tag="ex")
                nc.scalar.activation(ex[:], mn[:], mybir.ActivationFunctionType.Exp)
                h1r = psm.tile([128, HEADS * HID], F32, tag="h1r")
                nc.vector.scalar_tensor_tensor(
                    out=h1r[:], in0=x1[:], scalar=0.0, in1=ex[:],
                    op0=mybir.AluOpType.max, op1=mybir.AluOpType.add)

                # layer-2 node phase for this tile
                h1t_ps = pe_ps2.tile([128, 128], F32, tag="h1t")
                nc.tensor.transpose(out=h1t_ps[:], in_=h1r[:], identity=identity[:])
                h1T = psm.tile([128, 128], F32, tag="h1T")
                nc.scalar.copy(h1T[:], h1t_ps[:])
                a2_ps = pe_ps2.tile([128, HID + 2], F32, tag="a2")
                nc.tensor.matmul(a2_ps[:], lhsT=h1T[:], rhs=rhsW2[:],
                                 start=True, stop=True)
                a2s = psm.tile([128, HID + 2], F32, tag="a2s")
                nc.vector.tensor_tensor(out=a2s[:], in0=a2_ps[:], in1=c2_b[:],
                                        op=mybir.AluOpType.subtract)
                row2 = psm.tile([128, R2], BF16, tag="row2")
                nc.vector.memset(row2[:, HID + 2:], 0.0)
                nc.scalar.copy(row2[:, :HID], a2s[:, :HID])
                nc.vector.memset(row2[:, HID:HID + 1], 1.0)
                nc.scalar.copy(row2[:, HID + 1:HID + 2], a2s[:, HID:HID + 1])
                nc.gpsimd.indirect_dma_start(
                    out=tab2_loc.ap(),
                    out_offset=IndirectOffsetOnAxis(ap=oidx[:], axis=0),
                    in_=row2[:], in_offset=None)
                sd2 = psm.tile([128, RS], BF16, tag="sd2")
                nc.vector.memset(sd2[:, 1:], 0.0)
                nc.scalar.copy(sd2[:, :1], a2s[:, HID + 1:HID + 2])
                nc.gpsimd.indirect_dma_start(
                    out=sdst2.ap(),
                    out_offset=IndirectOffsetOnAxis(ap=oidx[:], axis=0),
                    in_=sd2[:], in_offset=None)

        nc.gpsimd.collective_compute(
            "AllGather", mybir.AluOpType.bypass, replica_groups=replica_groups,
            ins=[tab2_loc[:NS, :]], outs=[tab2.ap()])

        # ------------------------------------------------------------------
        # Phase E: edge phase layer 2 -> z
        # ------------------------------------------------------------------
        n_tiles_e = (min(n_tiles, cap3) if cap3 else n_tiles) if 3 in phases else 0
        tab2_q = [tab2[q * cfg.CH:(q + 1) * cfg.CH, :] for q in range(NQ)]
        with tc.tile_pool(name="qi", bufs=4) as qi, \
             tc.tile_pool(name="qg", bufs=3) as qg, \
             tc.tile_pool(name="qo", bufs=3) as qo, \
             tc.tile_pool(name="qs", bufs=3) as qs, \
             tc.tile_pool(name="qe_ps", bufs=4, space="PSUM") as qe_ps:
            for t in range(n_tiles_e):
                gi = qi.tile([128, NQ, cfg.QS // 16], I16, tag="gi2")
                nc.sync.dma_start(
                    out=gi[:], in_=gidx_in[t].rearrange("q p s -> p q s"))
                si = qi.tile([128, cfg.SLOTS // 16], I16, tag="si2")
                nc.sync.dma_start(out=si[:], in_=sidx_in[t])
                dloc = qi.tile([128, G], I32, tag="dloc2")
                nc.sync.dma_start(out=dloc[:], in_=dloc_in[t])
                oidx = qi.tile([128, 1], I32, tag="oidx2")
                nc.sync.dma_start(out=oidx[:], in_=oidx_in[t])

                hg2 = qg.tile([128, G, R2], BF16, tag="hg2")
                for q in range(NQ):
                    nc.gpsimd.dma_gather(
                        out_ap=hg2[:, q * cfg.NBQ:(q + 1) * cfg.NBQ, :],
                        in_ap=tab2_q[q],
                        idxs_ap=gi[:, q, :],
                        num_idxs=cfg.QS, num_idxs_reg=cfg.QS,
                        elem_size=R2)
                sde2 = qg.tile([128, G, RS], BF16, tag="sde2")
                nsp = cfg.SLOTS // 1024 if cfg.SLOTS > 1024 else 1
                bsp = G // nsp
                assert bsp * 128 <= 1024 and bsp * nsp == G, (cfg.SLOTS, G)
                for hsp in range(nsp):
                    nc.gpsimd.dma_gather(
                        out_ap=sde2[:, hsp * bsp:(hsp + 1) * bsp, :],
                        in_ap=sdst2[:NS, :],
                        idxs_ap=si[:, hsp * (bsp * 8):(hsp + 1) * (bsp * 8)],
                        num_idxs=bsp * 128, num_idxs_reg=bsp * 128, elem_size=RS)

                lg2 = qs.tile([128, G, 1], BF16, tag="lg2")
                nc.vector.tensor_add(lg2[:], hg2[:, :, HID + 1:HID + 2],
                                     sde2[:, :, :1])
                lr2 = qs.tile([128, G, 1], BF16, tag="lr2")
                nc.vector.scalar_tensor_tensor(
                    out=lr2[:], in0=lg2[:], scalar=float(NEG_SLOPE), in1=lg2[:],
                    op0=mybir.AluOpType.mult, op1=mybir.AluOpType.max)
                w2t = qs.tile([128, G, 1], BF16, tag="w2")
                nc.scalar.activation(w2t[:], lr2[:],
                                     mybir.ActivationFunctionType.Exp)

                oh = qo.tile([128, G, 128], BF16, tag="oh2")
                nc.vector.tensor_tensor(
                    out=oh[:], in0=dloc[:].to_broadcast([128, G, 128]),
                    in1=bcast_mid(iota_t[:], G), op=mybir.AluOpType.is_equal)

                rhs2 = qg.tile([128, G, HID + 1], BF16, tag="rhs2")
                nc.vector.tensor_mul(rhs2[:], hg2[:, :, :HID + 1],
                                     w2t[:].to_broadcast([128, G, HID + 1]))

                acc_ps = qe_ps.tile([128, HID + 1], F32, tag="accz")
                for g in range(G):
                    nc.tensor.matmul(acc_ps[:], lhsT=oh[:, g, :],
                                     rhs=rhs2[:, g, :],
                                     start=(g == 0), stop=(g == G - 1))

                den = qs.tile([128, 1], F32, tag="den2")
                nc.vector.tensor_scalar(out=den[:], in0=acc_ps[:, HID:HID + 1],
                                        scalar1=1e-20, scalar2=None,
                                        op0=mybir.AluOpType.add)
                rec = qs.tile([128, 1], F32, tag="rec2")
                nc.vector.reciprocal(rec[:], den[:])
                zt = qs.tile([128, HID], F32, tag="zt")
                nc.vector.tensor_scalar(out=zt[:], in0=acc_ps[:, :HID],
                                        scalar1=rec[:, :1], scalar2=None,
                                        op0=mybir.AluOpType.mult)
                nc.vector.tensor_add(zt[:], zt[:], b2_b[:])
                nc.gpsimd.indirect_dma_start(
                    out=z_out.ap(),
                    out_offset=IndirectOffsetOnAxis(ap=oidx[:], axis=0),
                    in_=zt[:], in_offset=None)

    nc.compile()
    return nc


# ---------------------------------------------------------------------------
# Entry point
# ---------------------------------------------------------------------------

def _run(inputs, cfg: Cfg, trace=False):
    from concourse.bass_utils import run_bass_kernel_spmd

    x = np.asarray(inputs["x"], dtype=np.float32)
    ei = np.asarray(inputs["edge_index"])
    W1 = np.asarray(inputs["W1"], dtype=np.float32)
    A1 = make_blockdiag(np.asarray(inputs["att_src1"], dtype=np.float32),
                        np.asarray(inputs["att_dst1"], dtype=np.float32))
    b1 = np.asarray(inputs["bias1"], dtype=np.float32).reshape(1, -1)
    W2 = np.asarray(inputs["W2"], dtype=np.float32)
    A2 = make_blockdiag(np.asarray(inputs["att_src2"], dtype=np.float32),
                        np.asarray(inputs["att_dst2"], dtype=np.float32))
    b2 = np.asarray(inputs["bias2"], dtype=np.float32).reshape(1, -1)

    pre = preprocess(ei, cfg)
    nc = build_program(cfg, pre["n_tiles"])

    in_maps = []
    for c in range(cfg.C):
        in_maps.append({
            "x_shard": np.ascontiguousarray(x[c * cfg.NS:(c + 1) * cfg.NS]),
            "W1": W1, "A1": A1, "bias1": b1,
            "W2": W2, "A2": A2, "bias2": b2,
            "gidx": np.ascontiguousarray(pre["gidx"][c]),
            "sidx": np.ascontiguousarray(pre["sidx"][c]),
            "dst_local": np.ascontiguousarray(pre["dst_local"][c]),
            "out_idx": np.ascontiguousarray(pre["out_idx"][c][:, :, None]),
        })
    res = run_bass_kernel_spmd(nc, in_maps, core_ids=list(range(cfg.C)),
                               trace=trace)
    z = np.concatenate([res.results[c]["z"][:cfg.NS] for c in range(cfg.C)], axis=0)
    return z, res


def kernel(**inputs) -> np.ndarray:
    z, _ = _run(inputs, Cfg())
    return z



# revision 2
# speedup vs baseline: 2940.0620x; 2940.0620x over previous
"""2-layer GAT (GATConv x2, PyG-style, eval mode) on 8 Trainium2 NeuronCores.

Design (dst-node sharding, fixed 128-node tiles, edge/graph parallelism):
  - Self-loops appended. Core c owns dst nodes [c*NS, (c+1)*NS); within a
    core, tile t owns dst nodes [128t, 128(t+1)) — the SAME relative range on
    every core, so every node-side write is a cheap contiguous DMA (data-
    dependent indirect scatters were 65% of device DMA time in the previous
    design).
  - Per layer, a node phase computes h = x @ W plus per-node attention logits
    (fused via a block-diagonal attention matrix). src-side rows [h | s_src]
    go to a bf16 gather table, AllGathered in 4 pieces so each piece's
    transfer overlaps the remaining node tiles and the first edge tiles.
    dst-side logits s_dst stay in a persistent SBUF tile — no DRAM table, no
    per-edge gather: a transposed one-hot matmul broadcasts them to slots.
  - Edge phase: edges grouped per (tile, src-chunk) into 128-edge blocks
    (chunking keeps bulk-gather indices int16). Rows are fetched with
    dma_gather spread over 4 SWDGE queues (descriptor scratch scaled to
    match). w = exp(leaky_relu(s_src + s_dst)); a 0/1 one-hot
    [slot, local_dst] (built on DVE; its transpose on the Activation engine
    as Relu(1 - Abs(id - p))) routes weighted messages into PSUM via TensorE
    matmuls. The softmax denominator rides along as appended w-columns in the
    same matmul, so normalization is one reciprocal+scale per node.
  - All per-core variation lives in data/index arrays so one SPMD program
    serves all 8 cores. `reps` replays the whole forward pass inside one
    NEFF for slope-based device timing.
"""

import math
import numpy as np

import concourse.bass as bass
import concourse.bacc as bacc
import concourse.tile as tile
from concourse import mybir
from concourse.bass import AP
from concourse.masks import make_identity

F32 = mybir.dt.float32
BF16 = mybir.dt.bfloat16
I16 = mybir.dt.int16
I8 = mybir.dt.int8

N_NODES = 100000
IN_CH = 128
HID = 32
HEADS = 4
NEG_SLOPE = 0.2
N_CORES = 8

NQ = 4                      # src chunks (table rows per chunk fit int16)
NS = N_NODES // N_CORES     # nodes per core (dst shard)
CH = N_NODES // NQ          # table chunk rows
T = math.ceil(NS / 128)     # fixed 128-node tiles per core
PIECE = NS // NQ            # shard piece per chunked AllGather
R1 = 256                    # tab1 row: [h(128) | s_src(4) | pad] bf16 (512B)
R2 = 128                    # tab2 row: [h2(32) | s2_src(1) | pad] bf16 (256B)
NQUEUES = 4


# ---------------------------------------------------------------------------
# Host-side preprocessing (vectorized)
# ---------------------------------------------------------------------------

def preprocess(edge_index):
    src = np.ascontiguousarray(edge_index[0]).astype(np.int64)
    dst = np.ascontiguousarray(edge_index[1]).astype(np.int64)
    loops = np.arange(N_NODES, dtype=np.int64)
    src = np.concatenate([src, loops]).astype(np.int32)
    dst = np.concatenate([dst, loops]).astype(np.int32)

    core = dst // NS
    rel = dst - core * NS
    t = rel >> 7
    # gather-table row of node n: (piece i, core c, j) with i = (n%NS)//PIECE
    # so chunk q == AllGather piece q; index within chunk = c*PIECE + j
    sc = src // NS
    sr = src - sc * NS
    q = sr // PIECE
    srcrel = sc * PIECE + (sr - q * PIECE)
    cell = ((core.astype(np.int64) * T + t) * NQ + q)

    order = np.argsort(cell, kind="stable")
    cs = cell[order]
    srcs = srcrel[order]
    rels = rel[order]

    n_cells = N_CORES * T * NQ
    cnt = np.bincount(cs, minlength=n_cells)
    starts = np.zeros(n_cells + 1, dtype=np.int64)
    np.cumsum(cnt, out=starts[1:])
    pos = np.arange(len(cs), dtype=np.int64) - starts[cs]

    nbq = int(math.ceil(cnt.max() / 128))
    qs = nbq * 128
    g = NQ * nbq

    gflat = np.zeros((n_cells, qs), np.int16)
    gflat[cs, pos] = srcs.astype(np.int16)
    # [C, T, 16, NQ*QS16]: per wrap-row, all chunks' index columns contiguous
    # so the on-device replicate-to-128-partitions DMA needs only 3 AP dims
    gidx = np.ascontiguousarray(
        gflat.reshape(N_CORES, T, NQ, qs // 16, 16).transpose(0, 1, 4, 2, 3)
        .reshape(N_CORES, T, 16, NQ * (qs // 16)))

    tile_id = cs // NQ
    blk = (cs % NQ) * nbq + pos // 128
    lane = pos % 128
    dlocp = np.full((N_CORES * T, 128, g), -1, np.int8)
    dlocp[tile_id, lane, blk] = (rels - (tile_id % T) * 128).astype(np.int8)
    dlocp = dlocp.reshape(N_CORES, T, 128, g)
    dlocsl = np.ascontiguousarray(dlocp.transpose(0, 1, 3, 2))
    return dict(gidx=gidx, dlocp=dlocp, dlocsl=dlocsl, nbq=nbq)


def make_blockdiag(att_src, att_dst):
    heads, hid = att_src.shape
    A = np.zeros((heads * hid, 2 * heads), dtype=np.float32)
    for h in range(heads):
        A[h * hid:(h + 1) * hid, h] = att_src[h]
        A[h * hid:(h + 1) * hid, heads + h] = att_dst[h]
    return A


# ---------------------------------------------------------------------------
# Device program
# ---------------------------------------------------------------------------

def bcast_mid(ap: AP, reps: int) -> AP:
    (p_step, p_num), rest = ap.ap[0], list(ap.ap[1:])
    return AP(tensor=ap.tensor, offset=ap.offset,
              ap=[[p_step, p_num], [0, reps]] + rest)


def build_program(nbq, reps=1):
    from contextlib import ExitStack
    # SWDGE descriptor scratch scales with queue count: each queue's ring must
    # hold at least one full gather call's descriptors or the device wedges
    nc = bacc.Bacc(None, target_bir_lowering=False, num_swdge_queues=NQUEUES,
                   dynamic_dma_scratch_size=16384 * NQUEUES)
    QS = nbq * 128
    QS16 = QS // 16
    G = NQ * nbq
    AW = 2 * HEADS

    # ---- IO ----
    xT_in = nc.dram_tensor("xT", [IN_CH, NS], F32, kind="ExternalInput")
    W1_in = nc.dram_tensor("W1", [IN_CH, HEADS * HID], F32, kind="ExternalInput")
    A1_in = nc.dram_tensor("A1", [HEADS * HID, AW], F32, kind="ExternalInput")
    b1_in = nc.dram_tensor("bias1", [1, HEADS * HID], F32, kind="ExternalInput")
    W2_in = nc.dram_tensor("W2", [HEADS * HID, HID], F32, kind="ExternalInput")
    A2_in = nc.dram_tensor("A2", [HID, 2], F32, kind="ExternalInput")
    b2_in = nc.dram_tensor("bias2", [1, HID], F32, kind="ExternalInput")
    gidx_in = nc.dram_tensor("gidx", [T, 16, NQ * QS16], I16, kind="ExternalInput")
    dlocp_in = nc.dram_tensor("dlocp", [T, 128, G], I8, kind="ExternalInput")
    dlocsl_in = nc.dram_tensor("dlocsl", [T, G, 128], I8, kind="ExternalInput")
    z_out = nc.dram_tensor("z", [NS, HID], F32, kind="ExternalOutput")

    # ---- internal DRAM ----
    tab1_loc = nc.dram_tensor("tab1_loc", [NS, R1], BF16)
    tab1 = nc.dram_tensor("tab1", [N_NODES, R1], BF16, addr_space="Shared")
    tab2_loc = nc.dram_tensor("tab2_loc", [NS, R2], BF16)
    tab2 = nc.dram_tensor("tab2", [N_NODES, R2], BF16, addr_space="Shared")
    scratch_c2 = nc.dram_tensor("scratch_c2", [1, HID + 2], F32)

    replica_groups = [list(range(N_CORES))]

    def nn_of(t):
        return min(128, NS - t * 128)

    with tile.TileContext(nc) as tc, ExitStack() as stack:
        consts = stack.enter_context(tc.tile_pool(name="consts", bufs=1))
        ppre_cm = tc.tile_pool(name="ppre", bufs=1, space="PSUM")
        ppre = ppre_cm.__enter__()

        identity = consts.tile([128, 128], F32)
        make_identity(nc, identity[:])
        iota_j8 = consts.tile([128, 128], I8)
        nc.gpsimd.iota(iota_j8[:], pattern=[[1, 128]], base=0,
                       channel_multiplier=0,
                       allow_small_or_imprecise_dtypes=True)
        # negated per-partition index (-p) as f32, bias operand for the
        # ACT-engine one-hot build: ohT = Relu(1 - Abs(A - p))
        negp = consts.tile([128, 1], F32)
        iota_c32 = consts.tile([128, 1], mybir.dt.int32)
        nc.gpsimd.iota(iota_c32[:], pattern=[[0, 1]], base=0,
                       channel_multiplier=1)
        nc.vector.tensor_scalar(out=negp[:], in0=iota_c32[:], scalar1=-1.0,
                                scalar2=None, op0=mybir.AluOpType.mult)

        # rhsW1 = [W1 | W1 @ A1]  [128, 136]
        rhsW1 = consts.tile([128, IN_CH + AW], F32)
        nc.sync.dma_start(out=rhsW1[:, :HEADS * HID], in_=W1_in[:])
        W1s = consts.tile([128, HEADS * HID], F32)
        nc.sync.dma_start(out=W1s[:], in_=W1_in[:])
        A1s = consts.tile([HEADS * HID, AW], F32)
        nc.sync.dma_start(out=A1s[:], in_=A1_in[:])
        w1t_ps = ppre.tile([128, 128], F32)
        nc.tensor.transpose(out=w1t_ps[:], in_=W1s[:], identity=identity[:])
        W1T = consts.tile([128, 128], F32)
        nc.scalar.copy(W1T[:], w1t_ps[:])
        w1a_ps = ppre.tile([128, AW], F32)
        nc.tensor.matmul(w1a_ps[:], lhsT=W1T[:], rhs=A1s[:], start=True, stop=True)
        nc.scalar.copy(rhsW1[:, IN_CH:], w1a_ps[:])

        # rhsW2 = [W2 | W2 @ A2]  [128, 34]
        rhsW2 = consts.tile([128, HID + 2], F32)
        W2s = consts.tile([128, HID], F32)
        nc.sync.dma_start(out=W2s[:], in_=W2_in[:])
        nc.sync.dma_start(out=rhsW2[:, :HID], in_=W2_in[:])
        A2s = consts.tile([HID, 2], F32)
        nc.sync.dma_start(out=A2s[:], in_=A2_in[:])
        w2t_ps = ppre.tile([HID, 128], F32)
        nc.tensor.transpose(out=w2t_ps[:], in_=W2s[:], identity=identity[:])
        W2T = consts.tile([HID, 128], F32)
        nc.scalar.copy(W2T[:], w2t_ps[:])
        w2a_ps = ppre.tile([128, 2], F32)
        nc.tensor.matmul(w2a_ps[:], lhsT=W2T[:], rhs=A2s[:], start=True, stop=True)
        nc.scalar.copy(rhsW2[:, HID:], w2a_ps[:])

        # c2 = column sums of rhsW2 (elu "-1" correction: true h1 = h1raw - 1)
        ones_col = consts.tile([128, 1], F32)
        nc.vector.memset(ones_col[:], 1.0)
        c2_ps = ppre.tile([1, HID + 2], F32)
        nc.tensor.matmul(c2_ps[:], lhsT=ones_col[:], rhs=rhsW2[:], start=True,
                         stop=True)
        c2_row = consts.tile([1, HID + 2], F32)
        nc.vector.tensor_copy(c2_row[:], c2_ps[:])
        nc.sync.dma_start(out=scratch_c2[:], in_=c2_row[:])
        c2_b = consts.tile([128, HID + 2], F32)
        nc.sync.dma_start(out=c2_b[:],
                          in_=scratch_c2.ap().to_broadcast([128, HID + 2]))

        b1_b = consts.tile([128, HEADS * HID], F32)
        nc.sync.dma_start(out=b1_b[:], in_=b1_in.ap().to_broadcast([128, HEADS * HID]))
        b2_b = consts.tile([128, HID], F32)
        nc.sync.dma_start(out=b2_b[:], in_=b2_in.ap().to_broadcast([128, HID]))

        # persistent dst-side logits (rewritten every rep)
        sdstS = consts.tile([128, T * HEADS], BF16)
        sdstS2 = consts.tile([128, T], BF16)

        ppre_cm.__exit__(None, None, None)

        tab1_q = [tab1[q * CH:(q + 1) * CH, :] for q in range(NQ)]
        tab2_q = [tab2[q * CH:(q + 1) * CH, :] for q in range(NQ)]

        def gathers(pool_out, tab_chunks, gi, elem):
            for q in range(NQ):
                for off in range(0, QS, 1024):
                    n = min(1024, QS - off)
                    nc.gpsimd.dma_gather(
                        out_ap=pool_out[:, q * nbq + off // 128:
                                        q * nbq + (off + n) // 128, :],
                        in_ap=tab_chunks[q],
                        idxs_ap=gi[:, q, off // 16:(off + n) // 16],
                        num_idxs=n, num_idxs_reg=n, elem_size=elem,
                        queue_num=q % NQUEUES)

        for _rep in range(reps):
            # --------------------------------------------------------------
            # Phase A: node phase layer 1 -> tab1_loc + sdstS
            # --------------------------------------------------------------
            with tc.tile_pool(name="pa", bufs=3) as pa, \
                 tc.tile_pool(name="pa_ps", bufs=2, space="PSUM") as pa_ps:
                for t in range(T):
                    lo = t * 128
                    nn = nn_of(t)
                    x_t = pa.tile([128, 128], F32, tag="x")
                    nc.sync.dma_start(out=x_t[:, :nn], in_=xT_in[:, lo:lo + nn])
                    if nn < 128:
                        nc.vector.memset(x_t[:, nn:], 0.0)
                    hs_ps = pa_ps.tile([128, IN_CH + AW], F32, tag="hs")
                    nc.tensor.matmul(hs_ps[:], lhsT=x_t[:], rhs=rhsW1[:],
                                     start=True, stop=True)
                    aug = pa.tile([128, R1], BF16, tag="aug")
                    nc.vector.tensor_copy(aug[:, :IN_CH], hs_ps[:, :IN_CH])
                    nc.scalar.copy(aug[:, IN_CH:IN_CH + HEADS],
                                   hs_ps[:, IN_CH:IN_CH + HEADS])
                    nc.scalar.copy(sdstS[:, t * HEADS:(t + 1) * HEADS],
                                   hs_ps[:, IN_CH + HEADS:IN_CH + 2 * HEADS])
                    nc.sync.dma_start(out=tab1_loc[lo:lo + nn, :], in_=aug[:nn])

            for q in range(NQ):
                nc.gpsimd.collective_compute(
                    "AllGather", mybir.AluOpType.bypass,
                    replica_groups=replica_groups,
                    ins=[tab1_loc[q * PIECE:(q + 1) * PIECE, :]],
                    outs=[tab1[q * CH:(q + 1) * CH, :]])

            # --------------------------------------------------------------
            # Phase C: edge phase layer 1 (+ fused layer-2 node phase)
            # --------------------------------------------------------------
            with tc.tile_pool(name="pi", bufs=3) as pi, \
                 tc.tile_pool(name="pg", bufs=3) as pg, \
                 tc.tile_pool(name="po", bufs=3) as po, \
                 tc.tile_pool(name="ps", bufs=3) as psm, \
                 tc.tile_pool(name="pe_ps", bufs=2, space="PSUM") as pe_ps, \
                 tc.tile_pool(name="pe_ps2", bufs=2, space="PSUM") as pe_ps2:
                for t in range(T):
                    lo = t * 128
                    nn = nn_of(t)
                    gi = pi.tile([128, NQ, QS16], I16, tag="gi")
                    nc.sync.dma_start(out=gi[:], in_=AP(
                        tensor=gidx_in, offset=t * 16 * NQ * QS16,
                        ap=[[0, 8], [NQ * QS16, 16], [1, NQ * QS16]]))
                    dlp = pi.tile([128, G], I8, tag="dlp")
                    nc.sync.dma_start(out=dlp[:], in_=dlocp_in[t])
                    Ab = pi.tile([128, G, 128], I8, tag="Ab")
                    nc.sync.dma_start(out=Ab[:], in_=AP(
                        tensor=dlocsl_in, offset=t * G * 128,
                        ap=[[0, 128], [1, G * 128]]))

                    hg = pg.tile([128, G, R1], BF16, tag="hg")
                    gathers(hg, tab1_q, gi, R1)

                    oh = po.tile([128, G, 128], BF16, tag="oh")
                    nc.vector.tensor_tensor(
                        out=oh[:], in0=dlp[:].to_broadcast([128, G, 128]),
                        in1=bcast_mid(iota_j8[:], G), op=mybir.AluOpType.is_equal)
                    ohT = po.tile([128, G, 128], BF16, tag="ohT")
                    nc.scalar.activation(ohT[:], Ab[:],
                                         mybir.ActivationFunctionType.Abs,
                                         bias=negp[:, :1])
                    nc.scalar.activation(ohT[:], ohT[:],
                                         mybir.ActivationFunctionType.Relu,
                                         bias=1.0, scale=-1.0)

                    # s_dst broadcast to slots: sde[s,g,h] = sum_d ohT[d,g,s]*sdst[d,h]
                    sde_ps = pe_ps.tile([128, G, HEADS], F32, tag="sde")
                    for g in range(G):
                        nc.tensor.matmul(
                            sde_ps[:, g, :], lhsT=ohT[:, g, :],
                            rhs=sdstS[:, t * HEADS:(t + 1) * HEADS],
                            start=True, stop=True)

                    # w = exp(leaky_relu(s_src + s_dst))
                    lg = psm.tile([128, G, HEADS], BF16, tag="lg")
                    nc.vector.tensor_add(lg[:], hg[:, :, IN_CH:IN_CH + HEADS],
                                         sde_ps[:])
                    lr = psm.tile([128, G, HEADS], BF16, tag="lr")
                    nc.vector.scalar_tensor_tensor(
                        out=lr[:], in0=lg[:], scalar=float(NEG_SLOPE), in1=lg[:],
                        op0=mybir.AluOpType.mult, op1=mybir.AluOpType.max)
                    w_t = psm.tile([128, G, HEADS], BF16, tag="w")
                    nc.scalar.activation(w_t[:], lr[:],
                                         mybir.ActivationFunctionType.Exp)

                    # weighted messages [h*w | w]
                    rhs_b = pg.tile([128, G, IN_CH + HEADS], BF16, tag="rhsb")
                    rhs_v = rhs_b[:, :, :IN_CH].rearrange(
                        "p g (h c) -> p g h c", h=HEADS, c=HID)
                    hg_v = hg[:, :, :IN_CH].rearrange(
                        "p g (h c) -> p g h c", h=HEADS, c=HID)
                    nc.vector.tensor_mul(rhs_v, hg_v,
                                         w_t[:].to_broadcast([128, G, HEADS, HID]))
                    nc.scalar.copy(rhs_b[:, :, IN_CH:], w_t[:])

                    acc_ps = pe_ps2.tile([128, IN_CH + HEADS], F32, tag="acc")
                    for g in range(G):
                        nc.tensor.matmul(acc_ps[:], lhsT=oh[:, g, :],
                                         rhs=rhs_b[:, g, :],
                                         start=(g == 0), stop=(g == G - 1))

                    # normalize + bias + elu -> h1raw (true h1 = h1raw - 1)
                    den = psm.tile([128, HEADS], F32, tag="den")
                    nc.vector.tensor_scalar(out=den[:], in0=acc_ps[:, IN_CH:],
                                            scalar1=1e-20, scalar2=None,
                                            op0=mybir.AluOpType.add)
                    rec = psm.tile([128, HEADS], F32, tag="rec")
                    nc.vector.reciprocal(rec[:], den[:])
                    x1 = psm.tile([128, HEADS * HID], F32, tag="x1")
                    x1_v = x1[:].rearrange("p (h c) -> p h c", h=HEADS, c=HID)
                    acc_v = acc_ps[:, :IN_CH].rearrange(
                        "p (h c) -> p h c", h=HEADS, c=HID)
                    nc.vector.tensor_mul(x1_v, acc_v,
                                         rec[:].to_broadcast([128, HEADS, HID]))
                    nc.vector.tensor_add(x1[:], x1[:], b1_b[:])
                    mn = psm.tile([128, HEADS * HID], F32, tag="mn")
                    nc.vector.tensor_scalar(out=mn[:], in0=x1[:], scalar1=0.0,
                                            scalar2=None, op0=mybir.AluOpType.min)
                    ex = psm.tile([128, HEADS * HID], F32, tag="ex")
                    nc.scalar.activation(ex[:], mn[:],
                                         mybir.ActivationFunctionType.Exp)
                    h1r = psm.tile([128, HEADS * HID], F32, tag="h1r")
                    nc.vector.scalar_tensor_tensor(
                        out=h1r[:], in0=x1[:], scalar=0.0, in1=ex[:],
                        op0=mybir.AluOpType.max, op1=mybir.AluOpType.add)

                    # layer-2 node phase for this tile
                    h1t_ps = pe_ps2.tile([128, 128], F32, tag="h1t")
                    nc.tensor.transpose(out=h1t_ps[:], in_=h1r[:],
                                        identity=identity[:])
                    h1T = psm.tile([128, 128], F32, tag="h1T")
                    nc.scalar.copy(h1T[:], h1t_ps[:])
                    a2_ps = pe_ps.tile([128, HID + 2], F32, tag="a2")
                    nc.tensor.matmul(a2_ps[:], lhsT=h1T[:], rhs=rhsW2[:],
                                     start=True, stop=True)
                    a2s = psm.tile([128, HID + 2], F32, tag="a2s")
                    nc.vector.tensor_tensor(out=a2s[:], in0=a2_ps[:], in1=c2_b[:],
                                            op=mybir.AluOpType.subtract)
                    row2 = psm.tile([128, R2], BF16, tag="row2")
                    nc.scalar.copy(row2[:, :HID + 1], a2s[:, :HID + 1])
                    nc.scalar.copy(sdstS2[:, t:t + 1], a2s[:, HID + 1:HID + 2])
                    nc.sync.dma_start(out=tab2_loc[lo:lo + nn, :], in_=row2[:nn])

            for q in range(NQ):
                nc.gpsimd.collective_compute(
                    "AllGather", mybir.AluOpType.bypass,
                    replica_groups=replica_groups,
                    ins=[tab2_loc[q * PIECE:(q + 1) * PIECE, :]],
                    outs=[tab2[q * CH:(q + 1) * CH, :]])

            # --------------------------------------------------------------
            # Phase E: edge phase layer 2 -> z
            # --------------------------------------------------------------
            with tc.tile_pool(name="qi", bufs=3) as qi, \
                 tc.tile_pool(name="qg", bufs=3) as qg, \
                 tc.tile_pool(name="qo", bufs=3) as qo, \
                 tc.tile_pool(name="qs", bufs=3) as qs, \
                 tc.tile_pool(name="qe_ps", bufs=2, space="PSUM") as qe_ps, \
                 tc.tile_pool(name="qe_ps2", bufs=2, space="PSUM") as qe_ps2:
                for t in range(T):
                    lo = t * 128
                    nn = nn_of(t)
                    gi = qi.tile([128, NQ, QS16], I16, tag="gi2")
                    nc.sync.dma_start(out=gi[:], in_=AP(
                        tensor=gidx_in, offset=t * 16 * NQ * QS16,
                        ap=[[0, 8], [NQ * QS16, 16], [1, NQ * QS16]]))
                    dlp = qi.tile([128, G], I8, tag="dlp2")
                    nc.sync.dma_start(out=dlp[:], in_=dlocp_in[t])
                    Ab = qi.tile([128, G, 128], I8, tag="Ab2")
                    nc.sync.dma_start(out=Ab[:], in_=AP(
                        tensor=dlocsl_in, offset=t * G * 128,
                        ap=[[0, 128], [1, G * 128]]))

                    hg2 = qg.tile([128, G, R2], BF16, tag="hg2")
                    gathers(hg2, tab2_q, gi, R2)

                    oh = qo.tile([128, G, 128], BF16, tag="oh2")
                    nc.vector.tensor_tensor(
                        out=oh[:], in0=dlp[:].to_broadcast([128, G, 128]),
                        in1=bcast_mid(iota_j8[:], G), op=mybir.AluOpType.is_equal)
                    ohT = qo.tile([128, G, 128], BF16, tag="ohT2")
                    nc.scalar.activation(ohT[:], Ab[:],
                                         mybir.ActivationFunctionType.Abs,
                                         bias=negp[:, :1])
                    nc.scalar.activation(ohT[:], ohT[:],
                                         mybir.ActivationFunctionType.Relu,
                                         bias=1.0, scale=-1.0)

                    sde_ps = qe_ps.tile([128, G, 1], F32, tag="sde2")
                    for g in range(G):
                        nc.tensor.matmul(sde_ps[:, g, :], lhsT=ohT[:, g, :],
                                         rhs=sdstS2[:, t:t + 1],
                                         start=True, stop=True)

                    lg2 = qs.tile([128, G, 1], BF16, tag="lg2")
                    nc.vector.tensor_add(lg2[:], hg2[:, :, HID:HID + 1], sde_ps[:])
                    lr2 = qs.tile([128, G, 1], BF16, tag="lr2")
                    nc.vector.scalar_tensor_tensor(
                        out=lr2[:], in0=lg2[:], scalar=float(NEG_SLOPE), in1=lg2[:],
                        op0=mybir.AluOpType.mult, op1=mybir.AluOpType.max)
                    w2t = qs.tile([128, G, 1], BF16, tag="w2")
                    nc.scalar.activation(w2t[:], lr2[:],
                                         mybir.ActivationFunctionType.Exp)

                    rhs2 = qg.tile([128, G, HID + 1], BF16, tag="rhs2")
                    nc.vector.tensor_mul(rhs2[:, :, :HID], hg2[:, :, :HID],
                                         w2t[:].to_broadcast([128, G, HID]))
                    nc.scalar.copy(rhs2[:, :, HID:], w2t[:])

                    acc_ps = qe_ps2.tile([128, HID + 1], F32, tag="accz")
                    for g in range(G):
                        nc.tensor.matmul(acc_ps[:], lhsT=oh[:, g, :],
                                         rhs=rhs2[:, g, :],
                                         start=(g == 0), stop=(g == G - 1))

                    den = qs.tile([128, 1], F32, tag="den2")
                    nc.vector.tensor_scalar(out=den[:], in0=acc_ps[:, HID:],
                                            scalar1=1e-20, scalar2=None,
                                            op0=mybir.AluOpType.add)
                    rec = qs.tile([128, 1], F32, tag="rec2")
                    nc.vector.reciprocal(rec[:], den[:])
                    zt = qs.tile([128, HID], F32, tag="zt")
                    nc.vector.tensor_mul(zt[:], acc_ps[:, :HID],
                                         rec[:].to_broadcast([128, HID]))
                    nc.vector.tensor_add(zt[:], zt[:], b2_b[:])
                    nc.sync.dma_start(out=z_out[lo:lo + nn, :], in_=zt[:nn])

    nc.compile()
    return nc


# ---------------------------------------------------------------------------
# Execution (PJRT via shard_map, device-resident input support)
# ---------------------------------------------------------------------------

_CTX_CACHE = {}


def _make_ctx(nbq, reps):
    key = (nbq, reps)
    if key in _CTX_CACHE:
        return _CTX_CACHE[key]
    import jax
    from jax.sharding import Mesh, PartitionSpec
    from jax.experimental.shard_map import shard_map
    from concourse import bass2jax
    bass2jax.install_neuronx_cc_hook()

    nc = build_program(nbq, reps=reps)
    partition_name = nc.partition_id_tensor.name if nc.partition_id_tensor else None
    in_names, out_names, out_avals, zero_shapes = [], [], [], []
    for alloc in nc.m.functions[0].allocations:
        if not isinstance(alloc, mybir.MemoryLocationSet):
            continue
        name = alloc.memorylocations[0].name
        if alloc.kind == "ExternalInput":
            if name != partition_name:
                in_names.append(name)
        elif alloc.kind == "ExternalOutput":
            shape = tuple(alloc.tensor_shape)
            dtype = mybir.dt.np(alloc.dtype)
            out_names.append(name)
            out_avals.append(jax.core.ShapedArray(shape, dtype))
            zero_shapes.append((shape, dtype))
    n_params, n_outs = len(in_names), len(out_avals)
    in_names_full = in_names + out_names + ([partition_name] if partition_name else [])

    def _body(*args):
        operands = list(args)
        if partition_name:
            operands.append(bass2jax.partition_id_tensor())
        outs = bass2jax._bass_exec_p.bind(
            *operands, out_avals=tuple(out_avals), in_names=tuple(in_names_full),
            out_names=tuple(out_names), lowering_input_output_aliases=(),
            sim_require_finite=True, sim_require_nnan=True, nc=nc)
        return tuple(outs)

    devices = jax.devices()[:N_CORES]
    mesh = Mesh(np.asarray(devices), ("core",))
    sharded = jax.jit(
        shard_map(_body, mesh=mesh,
                  in_specs=(PartitionSpec("core"),) * (n_params + n_outs),
                  out_specs=(PartitionSpec("core"),) * n_outs,
                  check_rep=False),
        donate_argnums=tuple(range(n_params, n_params + n_outs)),
        keep_unused=True)
    ctx = dict(nc=nc, sharded=sharded, in_names=in_names, out_names=out_names,
               zero_shapes=zero_shapes, mesh=mesh)
    _CTX_CACHE[key] = ctx
    return ctx


def _prep_inputs(inputs):
    x = np.asarray(inputs["x"], dtype=np.float32)
    ei = np.asarray(inputs["edge_index"])
    pre = preprocess(ei)
    A1 = make_blockdiag(np.asarray(inputs["att_src1"], dtype=np.float32),
                        np.asarray(inputs["att_dst1"], dtype=np.float32))
    A2 = make_blockdiag(np.asarray(inputs["att_src2"], dtype=np.float32),
                        np.asarray(inputs["att_dst2"], dtype=np.float32))
    xT = np.ascontiguousarray(x.T)
    in_maps = []
    for c in range(N_CORES):
        in_maps.append({
            "xT": np.ascontiguousarray(xT[:, c * NS:(c + 1) * NS]),
            "W1": np.asarray(inputs["W1"], np.float32), "A1": A1,
            "bias1": np.asarray(inputs["bias1"], np.float32).reshape(1, -1),
            "W2": np.asarray(inputs["W2"], np.float32), "A2": A2,
            "bias2": np.asarray(inputs["bias2"], np.float32).reshape(1, -1),
            "gidx": pre["gidx"][c], "dlocp": pre["dlocp"][c],
            "dlocsl": pre["dlocsl"][c]})
    return in_maps, pre["nbq"]


def _concat(ctx, in_maps):
    per_core = [[np.asarray(m[name]) for name in ctx["in_names"]] for m in in_maps]
    return [np.concatenate([per_core[c][i] for c in range(N_CORES)], axis=0)
            for i in range(len(ctx["in_names"]))]


def _zeros(ctx):
    return [np.zeros((N_CORES * s[0], *s[1:]), d) for s, d in ctx["zero_shapes"]]


def _prep_zeros(ctx):
    import jax
    from jax.sharding import NamedSharding, PartitionSpec
    sh = NamedSharding(ctx["mesh"], PartitionSpec("core"))
    dz = [jax.device_put(z, sh) for z in _zeros(ctx)]
    jax.block_until_ready(dz)
    return dz


def _launch(ctx, dev_in, dz=None):
    if dz is None:
        dz = _prep_zeros(ctx)
    return ctx["sharded"](*dev_in, *dz)


def _device_inputs(ctx, in_maps):
    import jax
    from jax.sharding import NamedSharding, PartitionSpec
    sh = NamedSharding(ctx["mesh"], PartitionSpec("core"))
    dev_in = [jax.device_put(a, sh) for a in _concat(ctx, in_maps)]
    jax.block_until_ready(dev_in)
    return dev_in


def _z_of(ctx, out):
    return np.asarray(out[ctx["out_names"].index("z")]).reshape(N_CORES * NS, HID)


def kernel(**inputs) -> np.ndarray:
    import jax
    in_maps, nbq = _prep_inputs(inputs)
    ctx = _make_ctx(nbq, 1)
    dev_in = _device_inputs(ctx, in_maps)
    out = _launch(ctx, dev_in)
    jax.block_until_ready(out)
    return _z_of(ctx, out)
